# revision 1
# baseline (speedup 1.0000x reference)
"""CapsNet dynamic-routing layer on 8 Trainium2 NeuronCores.

Math (per example, S=512 input capsules of dim D=256, 16 output capsules of
dim 32, O = 16*32 = 512):
  u_hat = x @ W                     # [S, O]
  b = 0; for 3 routing iters:
    c = softmax_n(b)                # over the 16-capsule axis
    s[n] = sum_s c[n,s] * u_hat[s, n*32:(n+1)*32]
    v = s / sqrt(|s|^2 + 1e-7)
    b[n,s] = v[n] . u_hat[s, n*32:(n+1)*32]
  out = v.flatten()

Sharding: pure data-parallel over the batch (256 examples -> 32 per core),
W replicated, no cross-core communication.

Per-core structure: examples are processed in groups of 4 so that all the
thin [16, *] routing tensors pack into 32-partition strips of full
128-partition tiles (strip j holds example 4g+j; rows 16..31 of each strip
are dead). The four per-example routing matmuls of a K-tile go to four
different PE column groups (tile_position=(0, 32j)) and run concurrently.

Layouts (per example):
  u_hat  [S, O]  (S on partitions, 4 tiles) - rhs of the s-matmul
  u_hatT [O, S]  (O on partitions, 4 tiles) - rhs of the b-update matmul
Both come straight off the tensor engine from xT = x.T (host-transposed)
since both contract over D. b is kept transposed as bT [S, 16-per-ex] so
softmax runs along the free dim. Matmul operands use float32r (single-pass
fp32, ~1.6e-4 relative) unless use_f32r=False.
"""

import sys

sys.path.insert(0, "/opt/trn_rl_repo")

import numpy as np

import concourse.bacc as bacc
import concourse.mybir as mybir
import concourse.tile as tile
from concourse import bass
from concourse.bass_utils import run_bass_kernel_spmd
from concourse.masks import make_identity

F32 = mybir.dt.float32
F32R = mybir.dt.float32r
BF16 = mybir.dt.bfloat16
U32 = mybir.dt.uint32
AX = mybir.AxisListType
AF = mybir.ActivationFunctionType
OP = mybir.AluOpType

B, S, D = 256, 512, 256
NC_, DC = 16, 32  # num_capsule, dim_capsule
O = NC_ * DC  # 512
N_CORES = 8
E = B // N_CORES  # 32 examples per core
G = 4  # examples per group (one per PE column-group strip)
ROUTINGS = 3
KT_D = D // 128  # 2 k-tiles over D
MT = 4  # 4 tiles over S and over O
RDT = F32R  # matmul operand dtype (set by build())
QMAGIC = 0x5F3759DF  # rsqrt seed magic


def host_masks():
    # dmask4[32j+n, n'*32+d] = (n' == n) for n < 16, else 0 (strip pads dead)
    dmask4 = np.zeros((128, O), np.float32)
    for j in range(G):
        for n in range(NC_):
            dmask4[32 * j + n, n * DC : (n + 1) * DC] = 1.0
    # vmaskL[q, j*64 + k*16 + n'] = (n' == 4k + q//32), j-independent
    vmaskL = np.zeros((128, G * 4 * NC_), np.float32)
    for q in range(128):
        for j in range(G):
            for k in range(4):
                vmaskL[q, j * 64 + k * NC_ + 4 * k + q // 32] = 1.0
    return dmask4, vmaskL


def emit_creation(nc, pools, consts, xT_ap, g, uh, uhT):
    (xp, up, utp, sp, ctp, pcre, pps, pet, pvt) = pools
    (W_t, dmask_t, vmask_t, ident_t, magic_t, c0_t) = consts

    # ---- load xT for 4 examples: [D, (e, S)] as 2 partition tiles ----
    xt = []
    for k in range(KT_D):
        t = xp.tile([128, G, S], RDT, tag=f"xt{k}")
        nc.sync.dma_start(
            t[:],
            xT_ap[G * g : G * (g + 1), 128 * k : 128 * (k + 1), :].rearrange(
                "e p s -> p e s"
            ),
        )
        xt.append(t)

    # ---- u_hat [S, O] and u_hatT [O, S] per example ----
    for j in range(G):
        for m in range(MT):
            pu = pcre.tile([128, O], F32, tag="pcre")
            for k in range(KT_D):
                nc.tensor.matmul(
                    pu[:],
                    xt[k][:, j, bass.ts(m, 128)],
                    W_t[k][:],
                    start=(k == 0),
                    stop=(k == KT_D - 1),
                )
            t = up.tile([128, O], RDT, tag=f"uh{j}{m}")
            nc.scalar.copy(t[:], pu[:])
            uh[j][m] = t
        yield
        for m in range(MT):
            pu = pcre.tile([128, S], F32, tag="pcre")
            for k in range(KT_D):
                nc.tensor.matmul(
                    pu[:],
                    W_t[k][:, bass.ts(m, 128)],
                    xt[k][:, j, :],
                    start=(k == 0),
                    stop=(k == KT_D - 1),
                )
            t = utp.tile([128, S], RDT, tag=f"uht{j}{m}")
            nc.scalar.copy(t[:], pu[:])
            uhT[j][m] = t
        yield


def emit_routing(nc, pools, consts, out_ap, g, uh, uhT):
    (xp, up, utp, sp, ctp, pcre, pps, pet, pvt) = pools
    (W_t, dmask_t, vmask_t, ident_t, magic_t, c0_t) = consts

    # ---- routing (4 examples packed in 32-partition strips) ----
    cT = None  # [S-chunk m] -> [128, (j,16)] compact; iter 0 uses 1/16 const
    v = None
    for it in range(ROUTINGS):
        # s_full_j = cT_j.T @ u_hat_j : own [16, O] psum per example, then a
        # fused mask+gather packs the strips into one [128, O] sbuf tile
        # (fp32-family matmuls cannot write PSUM at partition offset != 0)
        masked = sp.tile([128, O], F32, tag="masked")
        nc.gpsimd.memset(masked[:], 0.0)
        for j in range(G):
            ps = pps.tile([NC_, O], F32, tag="ps")
            for m in range(MT):
                lhs = (
                    c0_t[:]
                    if cT is None
                    else cT[m][:, NC_ * j : NC_ * (j + 1)]
                )
                nc.tensor.matmul(
                    ps[:],
                    lhs,
                    uh[j][m][:],
                    start=(m == 0),
                    stop=(m == MT - 1),
                )
            nc.vector.tensor_mul(
                masked[32 * j : 32 * j + NC_, :], ps[:], dmask_t[: NC_, :]
            )
        yield
        s = sp.tile([128, DC], F32, tag="s")
        nc.vector.tensor_reduce(
            s[:],
            masked[:].rearrange("p (n d) -> p d n", n=NC_),
            axis=AX.X,
            op=OP.add,
        )
        # squash: v = s * rsqrt(|s|^2 + 1e-7); rsqrt = quake seed + 3 Newton
        sq = sp.tile([128, DC], F32, tag="sq")
        ss = sp.tile([128, 1], F32, tag="ss")
        nc.scalar.activation(sq[:], s[:], AF.Square, accum_out=ss[:])
        q = sp.tile([128, 1], F32, tag="q")
        nc.vector.tensor_scalar_add(q[:], ss[:], 1e-7)
        sh = sp.tile([128, 1], U32, tag="sh")
        nc.vector.tensor_scalar(
            sh[:], q[:].bitcast(U32), 1, None, op0=OP.logical_shift_right
        )
        y = sp.tile([128, 1], F32, tag="y")
        nc.vector.tensor_tensor(
            y[:].bitcast(U32), magic_t[:], sh[:], op=OP.subtract
        )
        for _ in range(2):
            t2 = sp.tile([128, 1], F32, tag="t2")
            nc.vector.tensor_tensor(t2[:], y[:], y[:], op=OP.mult)
            nc.vector.tensor_tensor(t2[:], t2[:], q[:], op=OP.mult)
            nc.vector.tensor_scalar(
                t2[:], t2[:], -0.5, 1.5, op0=OP.mult, op1=OP.add
            )
            nc.vector.tensor_tensor(y[:], y[:], t2[:], op=OP.mult)
        v = sp.tile([128, DC], F32, tag="v")
        nc.vector.tensor_scalar_mul(v[:], s[:], y[:])

        if it == ROUTINGS - 1:
            break
        yield

        # ---- b update: bT'[strip j] = Vblk_j.T @ u_hatT_j ----
        vtp = pvt.tile([DC, 128], F32, tag="vtp")
        nc.tensor.transpose(vtp[:], v[:], ident_t[:])
        vv = sp.tile([128, G * 4], F32, tag="vv")
        vtp_jx = vtp[:].rearrange("p (j x) -> p j x", j=G)
        for r in range(4):  # strip row n_lo = r: VV[32r+d,(j,k)] = vtp[d,32j+4k+r]
            nc.vector.tensor_copy(
                vv[32 * r : 32 * (r + 1), :].rearrange("p (j k) -> p j k", j=G),
                vtp_jx[:, :, r : NC_ : 4],
            )
        vblk = sp.tile([128, G * 4 * NC_], RDT, tag="vblk")
        nc.vector.tensor_mul(
            vblk[:].rearrange("p (j k n) -> p j k n", j=G, k=4),
            vmask_t[:].rearrange("p (j k n) -> p j k n", j=G, k=4),
            vv[:]
            .rearrange("p (j k one) -> p j k one", j=G, one=1)
            .to_broadcast([128, G, 4, NC_]),
        )
        # bT'_j = Vblk_j.T @ u_hatT_j in its own [16, S] psum; the per-strip
        # exp packs results into one [128, S] sbuf tile
        expb = sp.tile([128, S], F32, tag="expb")
        nc.gpsimd.memset(expb[:], 0.0)
        for j in range(G):
            pb = pps.tile([NC_, S], F32, tag="ps")
            for k in range(MT):
                nc.tensor.matmul(
                    pb[:],
                    vblk[:, 64 * j + NC_ * k : 64 * j + NC_ * (k + 1)],
                    uhT[j][k][:],
                    start=(k == 0),
                    stop=(k == MT - 1),
                )
            nc.scalar.activation(
                expb[32 * j : 32 * j + NC_, :], pb[:], AF.Exp
            )
        yield
        et = pet.tile([128, MT, 128], F32, tag="et")
        r_all = sp.tile([128, MT * G], F32, tag="r_all")
        for m in range(MT):
            nc.tensor.transpose(et[:, m, :], expb[:, bass.ts(m, 128)], ident_t[:])
            nc.vector.tensor_reduce(
                r_all[:, G * m : G * (m + 1)],
                et[:, m, :].rearrange("p (j n) -> p j n", j=G)[:, :, :NC_],
                axis=AX.X,
                op=OP.add,
            )
        rinv = sp.tile([128, MT * G], F32, tag="rinv")
        for m in range(MT):
            nc.vector.reciprocal(
                rinv[:, G * m : G * (m + 1)], r_all[:, G * m : G * (m + 1)]
            )
        cT = []
        for m in range(MT):
            ct = ctp.tile([128, G * NC_], RDT, tag=f"ct{m}")
            nc.vector.tensor_mul(
                ct[:].rearrange("p (j n) -> p j n", j=G),
                et[:, m, :].rearrange("p (j n) -> p j n", j=G)[:, :, :NC_],
                rinv[:, G * m : G * (m + 1)]
                .rearrange("p (j one) -> p j one", one=1)
                .to_broadcast([128, G, NC_]),
            )
            cT.append(ct)

    # ---- output: strip j -> row 4g+j ----
    for j in range(G):
        nc.sync.dma_start(
            out_ap[G * g + j].rearrange("(n d) -> n d", n=NC_),
            v[32 * j : 32 * j + NC_, :],
        )


def build(n_ex=E, num_devices=N_CORES, use_f32r=True):
    global RDT
    RDT = F32R if use_f32r else F32
    assert n_ex % G == 0
    nc = bacc.Bacc(
        "TRN2", target_bir_lowering=False, debug=False, num_devices=num_devices
    )
    xT_d = nc.dram_tensor("xT", [n_ex, D, S], RDT, kind="ExternalInput")
    W_d = nc.dram_tensor("W", [D, O], RDT, kind="ExternalInput")
    dmask_d = nc.dram_tensor("dmask", [128, O], F32, kind="ExternalInput")
    vmask_d = nc.dram_tensor("vmask", [128, G * 4 * NC_], F32, kind="ExternalInput")
    out_d = nc.dram_tensor("out", [n_ex, O], F32, kind="ExternalOutput")

    with tile.TileContext(nc) as tc:
        with (
            tc.tile_pool(name="consts", bufs=1) as cp,
            tc.tile_pool(name="xp", bufs=2) as xp,
            tc.tile_pool(name="up", bufs=2) as up,
            tc.tile_pool(name="utp", bufs=2) as utp,
            tc.tile_pool(name="sp", bufs=4) as sp,
            tc.tile_pool(name="ctp", bufs=3) as ctp,
            tc.tile_pool(name="pcre", bufs=2, space=bass.MemorySpace.PSUM) as pcre,
            tc.tile_pool(name="pps", bufs=3, space=bass.MemorySpace.PSUM) as pps,
            tc.tile_pool(name="pet", bufs=2, space=bass.MemorySpace.PSUM) as pet,
            tc.tile_pool(name="pvt", bufs=1, space=bass.MemorySpace.PSUM) as pvt,
        ):
            W_t = []
            for k in range(KT_D):
                t = cp.tile([128, O], RDT, tag=f"W{k}")
                nc.sync.dma_start(t[:], W_d.ap()[128 * k : 128 * (k + 1), :])
                W_t.append(t)
            dmask_t = cp.tile([128, O], F32, tag="dmask")
            nc.sync.dma_start(dmask_t[:], dmask_d.ap())
            vmask_t = cp.tile([128, G * 4 * NC_], F32, tag="vmask")
            nc.sync.dma_start(vmask_t[:], vmask_d.ap())
            ident_t = cp.tile([128, 128], F32, tag="ident")
            make_identity(nc, ident_t[:])
            magic_t = cp.tile([128, 1], U32, tag="magic")
            nc.vector.memset(magic_t[:], QMAGIC)
            c0_t = cp.tile([128, NC_], RDT, tag="c0")
            c0_f = cp.tile([128, NC_], F32, tag="c0f")
            nc.vector.memset(c0_f[:], 1.0 / NC_)
            nc.vector.tensor_copy(c0_t[:], c0_f[:])

            pools = (xp, up, utp, sp, ctp, pcre, pps, pet, pvt)
            consts = (W_t, dmask_t, vmask_t, ident_t, magic_t, c0_t)
            ngroups = n_ex // G

            def creation_gen(g):
                uh = [[None] * MT for _ in range(G)]
                uhT = [[None] * MT for _ in range(G)]
                gen = emit_creation(nc, pools, consts, xT_d.ap(), g, uh, uhT)
                return gen, (uh, uhT)

            cgen, made = creation_gen(0)
            for _ in cgen:
                pass
            for g in range(ngroups):
                rgen = emit_routing(nc, pools, consts, out_d.ap(), g, *made)
                if g + 1 < ngroups:
                    cgen, made = creation_gen(g + 1)
                else:
                    cgen = None
                for _ in rgen:
                    if cgen is not None:
                        next(cgen, None)
                if cgen is not None:
                    for _ in cgen:
                        pass

    nc.compile()
    return nc


_cache = {}


def _get_program():
    if "nc" not in _cache:
        _cache["nc"] = build()
    return _cache["nc"]


def _run(x: np.ndarray, W: np.ndarray, **spmd_kwargs):
    x = np.asarray(x, np.float32)
    W = np.asarray(W, np.float32)
    nc = _get_program()
    xT = np.ascontiguousarray(x.transpose(0, 2, 1))  # [B, D, S]
    dmask, vmask = host_masks()
    in_maps = []
    for c in range(N_CORES):
        in_maps.append(
            {
                "xT": xT[c * E : (c + 1) * E],
                "W": W,
                "dmask": dmask,
                "vmask": vmask,
            }
        )
    res = run_bass_kernel_spmd(
        nc, in_maps, core_ids=list(range(N_CORES)), **spmd_kwargs
    )
    out = np.concatenate([res.results[c]["out"] for c in range(N_CORES)], axis=0)
    return out, res


def kernel(x: np.ndarray, W: np.ndarray) -> np.ndarray:
    return _run(x, W)[0]



# revision 4
# speedup vs baseline: 1.4490x; 1.4490x over previous
"""CapsNet dynamic-routing layer on 8 Trainium2 NeuronCores (v2).

Math (per example, S=512 input capsules of dim D=256, 16 output capsules of
dim 32, O = 16*32 = 512):
  u_hat = x @ W                     # [S, O]
  b = 0; for 3 routing iters:
    c = softmax_n(b)                # over the 16-capsule axis
    s[n] = sum_s c[n,s] * u_hat[s, n*32:(n+1)*32]
    v = s / sqrt(|s|^2 + 1e-7)
    b[n,s] = v[n] . u_hat[s, n*32:(n+1)*32]
  out = v.flatten()

Sharding: pure data-parallel over the batch (256 examples -> 32 per core),
W replicated, no cross-core communication.

v2 design (vs baseline):
- uhT = (x@W).T computed ONCE via matmul (W stationary, reused), u_hat
  obtained from uhT by PE transposes (fp16, 1 cyc/row) - halves creation PE.
- Routing matmuls (s = c@u_hat and b = v@u_hatT) use PE COLUMN TILING:
  the 4 examples of a group run concurrently in 4 32-column PE groups
  (tile_position=(0,32j), fp16 operands so PSUM partition offsets are legal).
  This turns 4 serial N=512 streams into ~1.
- Iteration 0 is folded into creation: c0 is uniform=1/16, so
  s0 = colsum(u_hat)/16 comes free from accum_out on the uhT psum->sbuf
  copies; squash runs in O-partition layout via two tiny matmuls
  (32-partition-group norm + partition-group broadcast).
- Routing operands in fp16 (~5e-4/elem); all accumulation fp32.
"""

import sys

sys.path.insert(0, "/opt/trn_rl_repo")

import numpy as np

import concourse.bacc as bacc
import concourse.mybir as mybir
import concourse.tile as tile
from concourse import bass
from concourse.bass_utils import run_bass_kernel_spmd

F32 = mybir.dt.float32
F32R = mybir.dt.float32r
F16 = mybir.dt.float16
AX = mybir.AxisListType
AF = mybir.ActivationFunctionType
OP = mybir.AluOpType

B, S, D = 256, 512, 256
NC_, DC = 16, 32  # num_capsule, dim_capsule
O = NC_ * DC  # 512
N_CORES = 8
E = B // N_CORES  # 32 examples per core
G = 4  # examples per group (one per PE column-group)
KT_D = D // 128  # 2 k-tiles over D
MT = 4  # 4 tiles over S and over O


def host_consts():
    # dmask[32j+n, n'*32+d] = (n' == n) for n < 16, else 0
    dmask = np.zeros((128, O), np.float32)
    for j in range(G):
        for n in range(NC_):
            dmask[32 * j + n, n * DC : (n + 1) * DC] = 1.0
    # vmask[p, j*128 + k*32 + n'] = (n' == 4k + p//32), n' in [0,32)
    vmask = np.zeros((128, G * 4 * DC), np.float16)
    for p in range(128):
        for j in range(G):
            for k in range(4):
                vmask[p, j * 128 + k * DC + 4 * k + p // 32] = 1.0
    identH = np.eye(128, dtype=np.float16)
    identF = np.eye(128, dtype=np.float32)
    ind4 = np.zeros((128, 4), np.float32)
    for p in range(128):
        ind4[p, p // 32] = 1.0
    ind4T = np.ascontiguousarray(ind4.T)
    return dmask, vmask, identH, identF, ind4, ind4T


def emit_creation(nc, pools, consts, xT_ap, g, uh, uhT, box):
    (xp, up, utp, sp, ctp, pcre, pps, ptb, psm) = pools
    (W_t, dmask_t, vmask_t, identH_t, identF_t, ind4_t, ind4T_t) = consts

    # ---- load xT for 4 examples: [D, (e, S)] as 2 partition tiles ----
    xt = []
    for k in range(KT_D):
        t = xp.tile([128, G, S], F32R, tag=f"xt{k}")
        nc.sync.dma_start(
            t[:],
            xT_ap[G * g : G * (g + 1), 128 * k : 128 * (k + 1), :].rearrange(
                "e p s -> p e s"
            ),
        )
        xt.append(t)

    # ---- u_hatT [O, S] per example, with colsum accumulated for iter 0 ----
    acc = sp.tile([128, NC_], F32, tag="acc")
    for j in range(G):
        for t in range(MT):
            pu = pcre.tile([128, S], F32, tag="pcre")
            for k in range(KT_D):
                nc.tensor.matmul(
                    pu[:],
                    W_t[k][:, bass.ts(t, 128)],
                    xt[k][:, j, :],
                    start=(k == 0),
                    stop=(k == KT_D - 1),
                )
            ut = utp.tile([128, S], F16, tag=f"uht{j}{t}")
            nc.any.tensor_scalar(
                ut[:],
                pu[:],
                1.0,
                None,
                op0=OP.mult,
                op1=OP.add,
                accum_out=acc[:, 4 * j + t : 4 * j + t + 1],
            )
            uhT[j][t] = ut
        yield

    # ---- u_hat [S, O] per example via PE transposes of uhT ----
    for j in range(G):
        for m in range(MT):
            tp = ptb.tile([128, O], F16, tag="tp")
            for t in range(MT):
                nc.tensor.transpose(
                    tp[:, bass.ts(t, 128)],
                    uhT[j][t][:, bass.ts(m, 128)],
                    identH_t[:],
                )
            u = up.tile([128, O], F16, tag=f"uh{j}{m}")
            nc.any.tensor_copy(u[:], tp[:])
            uh[j][m] = u
        yield

    # ---- fused iteration 0: v0 from colsums, in O-partition layout ----
    # s0 = acc/16; |s0|^2 per capsule via 32-partition-group-sum matmul
    sqa = sp.tile([128, NC_], F32, tag="sqa")
    nc.scalar.activation(sqa[:], acc[:], AF.Square)
    pn = psm.tile([128, 160], F32, tag="small")
    nc.tensor.matmul(pn[:4, 128:144], ind4_t[:], sqa[:], start=True, stop=True)
    q0 = sp.tile([4, NC_], F32, tag="q0")
    nc.vector.tensor_scalar(
        q0[:], pn[:4, 128:144], 1.0 / 256.0, 1e-7, op0=OP.mult, op1=OP.add
    )
    rq0 = sp.tile([4, NC_], F32, tag="rq0")
    nc.vector.reciprocal(rq0[:], q0[:])
    # f0 = sqrt(rq0)/16 ; v0 = acc * f0  (since v0 = (acc/16)*rsqrt(q0))
    f0 = sp.tile([4, NC_], F32, tag="f0")
    nc.scalar.activation(f0[:], rq0[:], AF.Sqrt, scale=1.0 / 256.0)
    nc.tensor.matmul(pn[:, 144:160], ind4T_t[:4, :], f0[:], start=True, stop=True)
    vv0 = sp.tile([128, NC_], F16, tag="vv0")
    nc.vector.tensor_tensor(vv0[:], acc[:], pn[:, 144:160], op=OP.mult)
    vblk0 = sp.tile([128, G * 4 * DC], F16, tag="vblk")
    nc.vector.tensor_mul(
        vblk0[:].rearrange("p (j k n) -> p j k n", j=G, k=4),
        vmask_t[:].rearrange("p (j k n) -> p j k n", j=G, k=4),
        vv0[:]
        .rearrange("p (j k one) -> p j k one", j=G, one=1)
        .to_broadcast([128, G, 4, DC]),
    )
    box[0] = vblk0
    yield


def emit_routing(nc, pools, consts, out_ap, g, uh, uhT, box):
    (xp, up, utp, sp, ctp, pcre, pps, ptb, psm) = pools
    (W_t, dmask_t, vmask_t, identH_t, identF_t, ind4_t, ind4T_t) = consts

    vblk = box[0]
    for it in range(2):
        # ---- b update: pb[32j+n, s] = v.u_hat, 4 examples in 4 col-groups --
        pb = pps.tile([128, S], F32, tag="ps")
        for j in range(G):
            for k in range(MT):
                nc.tensor.matmul(
                    pb[32 * j : 32 * j + 32, :],
                    vblk[:, 128 * j + DC * k : 128 * j + DC * (k + 1)],
                    uhT[j][k][:],
                    start=(k == 0),
                    stop=(k == MT - 1),
                    tile_position=(0, 32 * j),
                )
        yield
        expb = sp.tile([128, S], F16, tag="expb")
        nc.scalar.activation(expb[:], pb[:], AF.Exp)
        et = ptb.tile([128, S], F16, tag="tp")
        for m in range(MT):
            nc.tensor.transpose(
                et[:, bass.ts(m, 128)], expb[:, bass.ts(m, 128)], identH_t[:]
            )
        # softmax over the 16 live columns of each 32-strip
        et_v = et[:].rearrange("p (m j n) -> p m j n", m=MT, j=G)[:, :, :, :NC_]
        r_all = sp.tile([128, MT * G], F32, tag="r_all")
        nc.vector.tensor_reduce(
            r_all[:].rearrange("p (m j) -> p m j", m=MT), et_v, axis=AX.X, op=OP.add
        )
        rinv = sp.tile([128, MT * G], F32, tag="rinv")
        nc.vector.reciprocal(rinv[:], r_all[:])
        ct = ctp.tile([128, MT * G * DC], F16, tag="ct")
        nc.vector.tensor_mul(
            ct[:].rearrange("p (m j n) -> p m j n", m=MT, j=G),
            et[:].rearrange("p (m j n) -> p m j n", m=MT, j=G),
            rinv[:]
            .rearrange("p (m j one) -> p m j one", m=MT, one=1)
            .to_broadcast([128, MT, G, DC]),
        )
        yield
        # ---- s matmul: 4 examples in 4 col-groups, accumulate over m ----
        ps = pps.tile([128, O], F32, tag="ps")
        for j in range(G):
            for m in range(MT):
                nc.tensor.matmul(
                    ps[32 * j : 32 * j + 32, :],
                    ct[:, 128 * m + DC * j : 128 * m + DC * (j + 1)],
                    uh[j][m][:],
                    start=(m == 0),
                    stop=(m == MT - 1),
                    tile_position=(0, 32 * j),
                )
        yield
        # ---- extract block-diagonal -> s [strip, d], then squash ----
        masked = sp.tile([128, O], F32, tag="masked")
        nc.vector.tensor_mul(masked[:], ps[:], dmask_t[:])
        s = sp.tile([128, DC], F32, tag="s")
        nc.vector.tensor_reduce(
            s[:],
            masked[:].rearrange("p (n d) -> p d n", n=NC_),
            axis=AX.X,
            op=OP.add,
        )
        sq2 = sp.tile([128, DC], F32, tag="sq2")
        ss = sp.tile([128, 1], F32, tag="ss")
        nc.scalar.activation(sq2[:], s[:], AF.Square, accum_out=ss[:])
        q2 = sp.tile([128, 1], F32, tag="q2")
        nc.vector.tensor_scalar_add(q2[:], ss[:], 1e-7)
        rq2 = sp.tile([128, 1], F32, tag="rq2")
        nc.vector.reciprocal(rq2[:], q2[:])
        y = sp.tile([128, 1], F32, tag="y")
        nc.scalar.activation(y[:], rq2[:], AF.Sqrt)
        v = sp.tile([128, DC], F32, tag="v")
        nc.vector.tensor_scalar_mul(v[:], s[:], y[:])

        if it == 0:
            # ---- rebuild vblk from v (strip layout -> O layout) ----
            pv = psm.tile([128, 160], F32, tag="small")
            nc.tensor.transpose(pv[:DC, :128], v[:], identF_t[:])
            vv = sp.tile([128, NC_], F16, tag="vv")
            vtp_jx = pv[:DC, :128].rearrange("p (j x) -> p j x", j=G)
            for r in range(4):
                nc.vector.tensor_copy(
                    vv[32 * r : 32 * (r + 1), :].rearrange(
                        "p (j k) -> p j k", j=G
                    ),
                    vtp_jx[:, :, r : NC_ : 4],
                )
            vblk = sp.tile([128, G * 4 * DC], F16, tag="vblk")
            nc.vector.tensor_mul(
                vblk[:].rearrange("p (j k n) -> p j k n", j=G, k=4),
                vmask_t[:].rearrange("p (j k n) -> p j k n", j=G, k=4),
                vv[:]
                .rearrange("p (j k one) -> p j k one", j=G, one=1)
                .to_broadcast([128, G, 4, DC]),
            )
            yield

    # ---- output: strip j -> row 4g+j ----
    for j in range(G):
        nc.sync.dma_start(
            out_ap[G * g + j].rearrange("(n d) -> n d", n=NC_),
            v[32 * j : 32 * j + NC_, :],
        )


def build(n_ex=E, num_devices=N_CORES):
    assert n_ex % G == 0
    nc = bacc.Bacc(
        "TRN2", target_bir_lowering=False, debug=False, num_devices=num_devices
    )
    xT_d = nc.dram_tensor("xT", [n_ex, D, S], F32R, kind="ExternalInput")
    W_d = nc.dram_tensor("W", [D, O], F32R, kind="ExternalInput")
    dmask_d = nc.dram_tensor("dmask", [128, O], F32, kind="ExternalInput")
    vmask_d = nc.dram_tensor("vmask", [128, G * 4 * DC], F16, kind="ExternalInput")
    identH_d = nc.dram_tensor("identH", [128, 128], F16, kind="ExternalInput")
    identF_d = nc.dram_tensor("identF", [128, 128], F32, kind="ExternalInput")
    ind4_d = nc.dram_tensor("ind4", [128, 4], F32, kind="ExternalInput")
    ind4T_d = nc.dram_tensor("ind4T", [4, 128], F32, kind="ExternalInput")
    out_d = nc.dram_tensor("out", [n_ex, O], F32, kind="ExternalOutput")

    with tile.TileContext(nc) as tc:
        with (
            tc.tile_pool(name="consts", bufs=1) as cp,
            tc.tile_pool(name="xp", bufs=2) as xp,
            tc.tile_pool(name="up", bufs=2) as up,
            tc.tile_pool(name="utp", bufs=2) as utp,
            tc.tile_pool(name="sp", bufs=3) as sp,
            tc.tile_pool(name="ctp", bufs=3) as ctp,
            tc.tile_pool(name="pcre", bufs=2, space=bass.MemorySpace.PSUM) as pcre,
            tc.tile_pool(name="pps", bufs=2, space=bass.MemorySpace.PSUM) as pps,
            tc.tile_pool(name="ptb", bufs=3, space=bass.MemorySpace.PSUM) as ptb,
            tc.tile_pool(name="psm", bufs=1, space=bass.MemorySpace.PSUM) as psm,
        ):
            W_t = []
            for k in range(KT_D):
                t = cp.tile([128, O], F32R, tag=f"W{k}")
                nc.sync.dma_start(t[:], W_d.ap()[128 * k : 128 * (k + 1), :])
                W_t.append(t)
            dmask_t = cp.tile([128, O], F32, tag="dmask")
            nc.sync.dma_start(dmask_t[:], dmask_d.ap())
            vmask_t = cp.tile([128, G * 4 * DC], F16, tag="vmask")
            nc.sync.dma_start(vmask_t[:], vmask_d.ap())
            identH_t = cp.tile([128, 128], F16, tag="identH")
            nc.sync.dma_start(identH_t[:], identH_d.ap())
            identF_t = cp.tile([128, 128], F32, tag="identF")
            nc.sync.dma_start(identF_t[:], identF_d.ap())
            ind4_t = cp.tile([128, 4], F32, tag="ind4")
            nc.sync.dma_start(ind4_t[:], ind4_d.ap())
            ind4T_t = cp.tile([4, 128], F32, tag="ind4T")
            nc.sync.dma_start(ind4T_t[:4, :], ind4T_d.ap())

            pools = (xp, up, utp, sp, ctp, pcre, pps, ptb, psm)
            consts = (W_t, dmask_t, vmask_t, identH_t, identF_t, ind4_t, ind4T_t)
            ngroups = n_ex // G

            def creation_gen(g):
                uh = [[None] * MT for _ in range(G)]
                uhT = [[None] * MT for _ in range(G)]
                box = [None]
                gen = emit_creation(nc, pools, consts, xT_d.ap(), g, uh, uhT, box)
                return gen, (uh, uhT, box)

            cgen, made = creation_gen(0)
            for _ in cgen:
                pass
            for g in range(ngroups):
                rgen = emit_routing(nc, pools, consts, out_d.ap(), g, *made)
                if g + 1 < ngroups:
                    cgen, made = creation_gen(g + 1)
                else:
                    cgen = None
                for _ in rgen:
                    if cgen is not None:
                        next(cgen, None)
                if cgen is not None:
                    for _ in cgen:
                        pass

    nc.compile()
    return nc


_cache = {}


def _get_program():
    if "nc" not in _cache:
        _cache["nc"] = build()
    return _cache["nc"]


def _run(x: np.ndarray, W: np.ndarray, **spmd_kwargs):
    x = np.asarray(x, np.float32)
    W = np.asarray(W, np.float32)
    nc = _get_program()
    xT = np.ascontiguousarray(x.transpose(0, 2, 1))  # [B, D, S]
    dmask, vmask, identH, identF, ind4, ind4T = host_consts()
    in_maps = []
    for c in range(N_CORES):
        in_maps.append(
            {
                "xT": xT[c * E : (c + 1) * E],
                "W": W,
                "dmask": dmask,
                "vmask": vmask,
                "identH": identH,
                "identF": identF,
                "ind4": ind4,
                "ind4T": ind4T,
            }
        )
    res = run_bass_kernel_spmd(
        nc, in_maps, core_ids=list(range(N_CORES)), **spmd_kwargs
    )
    out = np.concatenate([res.results[c]["out"] for c in range(N_CORES)], axis=0)
    return out, res


def kernel(x: np.ndarray, W: np.ndarray) -> np.ndarray:
    return _run(x, W)[0]


# revision 6
# speedup vs baseline: 1.5770x; 1.0884x over previous
"""CapsNet dynamic-routing layer on 8 Trainium2 NeuronCores (v2).

Math (per example, S=512 input capsules of dim D=256, 16 output capsules of
dim 32, O = 16*32 = 512):
  u_hat = x @ W                     # [S, O]
  b = 0; for 3 routing iters:
    c = softmax_n(b)                # over the 16-capsule axis
    s[n] = sum_s c[n,s] * u_hat[s, n*32:(n+1)*32]
    v = s / sqrt(|s|^2 + 1e-7)
    b[n,s] = v[n] . u_hat[s, n*32:(n+1)*32]
  out = v.flatten()

Sharding: pure data-parallel over the batch (256 examples -> 32 per core),
W replicated, no cross-core communication.

v2 design (vs baseline):
- uhT = (x@W).T computed ONCE via matmul (W stationary, reused), u_hat
  obtained from uhT by PE transposes (fp16, 1 cyc/row) - halves creation PE.
- Routing matmuls (s = c@u_hat and b = v@u_hatT) use PE COLUMN TILING:
  the 4 examples of a group run concurrently in 4 32-column PE groups
  (tile_position=(0,32j), fp16 operands so PSUM partition offsets are legal).
  This turns 4 serial N=512 streams into ~1.
- Iteration 0 is folded into creation: c0 is uniform=1/16, so
  s0 = colsum(u_hat)/16 comes free from accum_out on the uhT psum->sbuf
  copies; squash runs in O-partition layout via two tiny matmuls
  (32-partition-group norm + partition-group broadcast).
- Routing operands in fp16 (~5e-4/elem); all accumulation fp32.
"""

import sys

sys.path.insert(0, "/opt/trn_rl_repo")

import numpy as np

import concourse.bacc as bacc
import concourse.mybir as mybir
import concourse.tile as tile
from concourse import bass
from concourse.bass_utils import run_bass_kernel_spmd

F32 = mybir.dt.float32
F32R = mybir.dt.float32r
F16 = mybir.dt.float16
U32 = mybir.dt.uint32
QMAGIC = 0x5F3759DF  # quake rsqrt seed
AX = mybir.AxisListType
AF = mybir.ActivationFunctionType
OP = mybir.AluOpType

B, S, D = 256, 512, 256
NC_, DC = 16, 32  # num_capsule, dim_capsule
O = NC_ * DC  # 512
N_CORES = 8
E = B // N_CORES  # 32 examples per core
G = 4  # examples per group (one per PE column-group)
KT_D = D // 128  # 2 k-tiles over D
MT = 4  # 4 tiles over S and over O


def host_consts():
    # dmask[32j+n, n'*32+d] = (n' == n) for n < 16, else 0
    dmask = np.zeros((128, O), np.float32)
    for j in range(G):
        for n in range(NC_):
            dmask[32 * j + n, n * DC : (n + 1) * DC] = 1.0
    # vmask[p, j*128 + k*32 + n'] = (n' == 4k + p//32), n' in [0,32)
    vmask = np.zeros((128, G * 4 * DC), np.float16)
    for p in range(128):
        for j in range(G):
            for k in range(4):
                vmask[p, j * 128 + k * DC + 4 * k + p // 32] = 1.0
    identH = np.eye(128, dtype=np.float16)
    identF = np.eye(128, dtype=np.float32)
    ind4 = np.zeros((128, 4), np.float32)
    for p in range(128):
        ind4[p, p // 32] = 1.0
    ind4T = np.ascontiguousarray(ind4.T)
    return dmask, vmask, identH, identF, ind4, ind4T


def quake_rsqrt(nc, sp, q, magic, P, N, tag):
    """y ~= rsqrt(q) via quake seed + 2 Newton steps (all DVE, no ACT table)."""
    sh = sp.tile([P, N], U32, tag=f"{tag}_sh")
    nc.vector.tensor_scalar(
        sh[:P, :], q.bitcast(U32), 1, None, op0=OP.logical_shift_right
    )
    y = sp.tile([P, N], F32, tag=f"{tag}_y")
    nc.vector.tensor_tensor(
        y[:P, :].bitcast(U32), magic[:P, :N], sh[:P, :], op=OP.subtract
    )
    for i in range(2):
        t2 = sp.tile([P, N], F32, tag=f"{tag}_t{i}")
        nc.vector.tensor_tensor(t2[:P, :], y[:P, :], y[:P, :], op=OP.mult)
        nc.vector.tensor_tensor(t2[:P, :], t2[:P, :], q, op=OP.mult)
        nc.vector.tensor_scalar(
            t2[:P, :], t2[:P, :], -0.5, 1.5, op0=OP.mult, op1=OP.add
        )
        nc.vector.tensor_tensor(y[:P, :], y[:P, :], t2[:P, :], op=OP.mult)
    return y


def emit_creation(nc, pools, consts, xT_ap, g, uh, uhT, box):
    (xp, up, utp, sp, ctp, pcre, pps, ptb, psm) = pools
    (W_t, dmask_t, vmask_t, identH_t, identF_t, ind4_t, ind4T_t, magic_t) = consts

    # ---- load xT for 4 examples: [D, (e, S)] as 2 partition tiles ----
    xt = []
    for k in range(KT_D):
        t = xp.tile([128, G, S], F32R, tag=f"xt{k}")
        nc.sync.dma_start(
            t[:],
            xT_ap[G * g : G * (g + 1), 128 * k : 128 * (k + 1), :].rearrange(
                "e p s -> p e s"
            ),
        )
        xt.append(t)

    # ---- u_hatT [O, S] per example, with colsum accumulated for iter 0 ----
    acc = sp.tile([128, NC_], F32, tag="acc")
    for j in range(G):
        for t in range(MT):
            pu = pcre.tile([128, S], F32, tag="pcre")
            for k in range(KT_D):
                nc.tensor.matmul(
                    pu[:],
                    W_t[k][:, bass.ts(t, 128)],
                    xt[k][:, j, :],
                    start=(k == 0),
                    stop=(k == KT_D - 1),
                )
            ut = utp.tile([128, S], F16, tag=f"uht{j}{t}")
            nc.scalar.activation(
                ut[:],
                pu[:],
                AF.Copy,
                accum_out=acc[:, 4 * j + t : 4 * j + t + 1],
            )
            uhT[j][t] = ut
        yield

    # ---- u_hat [S, O] per example via PE transposes of uhT ----
    for j in range(G):
        for m in range(MT):
            tp = ptb.tile([128, O], F16, tag="tp")
            for t in range(MT):
                nc.tensor.transpose(
                    tp[:, bass.ts(t, 128)],
                    uhT[j][t][:, bass.ts(m, 128)],
                    identH_t[:],
                )
            u = up.tile([128, O], F16, tag=f"uh{j}{m}")
            nc.vector.tensor_copy(u[:], tp[:])
            uh[j][m] = u
        yield

    # ---- fused iteration 0: v0 from colsums, in O-partition layout ----
    # s0 = acc/16; |s0|^2 per capsule via 32-partition-group-sum matmul
    sqa = sp.tile([128, NC_], F32, tag="sqa")
    nc.vector.tensor_tensor(sqa[:], acc[:], acc[:], op=OP.mult)
    pn = psm.tile([128, 160], F32, tag="small")
    nc.tensor.matmul(pn[:4, 128:144], ind4_t[:], sqa[:], start=True, stop=True)
    q0 = sp.tile([4, NC_], F32, tag="q0")
    nc.vector.tensor_scalar(
        q0[:], pn[:4, 128:144], 1.0 / 256.0, 1e-7, op0=OP.mult, op1=OP.add
    )
    # f0 = rsqrt(q0)/16 ; v0 = acc * f0  (since v0 = (acc/16)*rsqrt(q0))
    y0 = quake_rsqrt(nc, sp, q0[:], magic_t, 4, NC_, "q0")
    f0 = sp.tile([4, NC_], F32, tag="f0")
    nc.vector.tensor_scalar_mul(f0[:], y0[:4, :], 1.0 / 16.0)
    nc.tensor.matmul(pn[:, 144:160], ind4T_t[:4, :], f0[:], start=True, stop=True)
    vv0 = sp.tile([128, NC_], F16, tag="vv0")
    nc.vector.tensor_tensor(vv0[:], acc[:], pn[:, 144:160], op=OP.mult)
    vblk0 = sp.tile([128, G * 4 * DC], F16, tag="vblk")
    nc.gpsimd.tensor_mul(
        vblk0[:].rearrange("p (j k n) -> p j k n", j=G, k=4),
        vmask_t[:].rearrange("p (j k n) -> p j k n", j=G, k=4),
        vv0[:]
        .rearrange("p (j k one) -> p j k one", j=G, one=1)
        .to_broadcast([128, G, 4, DC]),
    )
    box[0] = vblk0
    yield


def emit_routing(nc, pools, consts, out_ap, g, uh, uhT, box):
    (xp, up, utp, sp, ctp, pcre, pps, ptb, psm) = pools
    (W_t, dmask_t, vmask_t, identH_t, identF_t, ind4_t, ind4T_t, magic_t) = consts

    vblk = box[0]
    for it in range(2):
        # ---- b update: pb[32j+n, s] = v.u_hat, 4 examples in 4 col-groups --
        pb = pps.tile([128, S], F32, tag="ps")
        for j in range(G):
            for k in range(MT):
                nc.tensor.matmul(
                    pb[32 * j : 32 * j + 32, :],
                    vblk[:, 128 * j + DC * k : 128 * j + DC * (k + 1)],
                    uhT[j][k][:],
                    start=(k == 0),
                    stop=(k == MT - 1),
                    tile_position=(0, 32 * j),
                )
        yield
        expb = sp.tile([128, S], F16, tag="expb")
        nc.scalar.activation(expb[:], pb[:], AF.Exp)
        et = ptb.tile([128, S], F16, tag="tp")
        for m in range(MT):
            nc.tensor.transpose(
                et[:, bass.ts(m, 128)], expb[:, bass.ts(m, 128)], identH_t[:]
            )
        # softmax over the 16 live columns of each 32-strip
        et_v = et[:].rearrange("p (m j n) -> p m j n", m=MT, j=G)[:, :, :, :NC_]
        r_all = sp.tile([128, MT * G], F32, tag="r_all")
        nc.vector.tensor_reduce(
            r_all[:].rearrange("p (m j) -> p m j", m=MT), et_v, axis=AX.X, op=OP.add
        )
        rinv = sp.tile([128, MT * G], F32, tag="rinv")
        nc.vector.reciprocal(rinv[:], r_all[:])
        ct = ctp.tile([128, MT * G * DC], F16, tag="ct")
        nc.vector.tensor_mul(
            ct[:].rearrange("p (m j n) -> p m j n", m=MT, j=G),
            et[:].rearrange("p (m j n) -> p m j n", m=MT, j=G),
            rinv[:]
            .rearrange("p (m j one) -> p m j one", m=MT, one=1)
            .to_broadcast([128, MT, G, DC]),
        )
        yield
        # ---- s matmul: 4 examples in 4 col-groups, accumulate over m ----
        ps = pps.tile([128, O], F32, tag="ps")
        for j in range(G):
            for m in range(MT):
                nc.tensor.matmul(
                    ps[32 * j : 32 * j + 32, :],
                    ct[:, 128 * m + DC * j : 128 * m + DC * (j + 1)],
                    uh[j][m][:],
                    start=(m == 0),
                    stop=(m == MT - 1),
                    tile_position=(0, 32 * j),
                )
        yield
        # ---- extract block-diagonal -> s [strip, d], then squash ----
        masked = sp.tile([128, O], F32, tag="masked")
        nc.vector.tensor_mul(masked[:], ps[:], dmask_t[:])
        s = sp.tile([128, DC], F32, tag="s")
        nc.vector.tensor_reduce(
            s[:],
            masked[:].rearrange("p (n d) -> p d n", n=NC_),
            axis=AX.X,
            op=OP.add,
        )
        sq2 = sp.tile([128, DC], F32, tag="sq2")
        nc.vector.tensor_tensor(sq2[:], s[:], s[:], op=OP.mult)
        q2 = sp.tile([128, 1], F32, tag="q2")
        nc.vector.tensor_reduce(q2[:], sq2[:], axis=AX.X, op=OP.add)
        nc.vector.tensor_scalar_add(q2[:], q2[:], 1e-7)
        y = quake_rsqrt(nc, sp, q2[:], magic_t, 128, 1, "q2")
        v = sp.tile([128, DC], F32, tag="v")
        nc.vector.tensor_scalar_mul(v[:], s[:], y[:])

        if it == 0:
            # ---- rebuild vblk from v (strip layout -> O layout) ----
            pv = psm.tile([128, 160], F32, tag="small")
            nc.tensor.transpose(pv[:DC, :128], v[:], identF_t[:])
            vv = sp.tile([128, NC_], F16, tag="vv")
            vtp_jx = pv[:DC, :128].rearrange("p (j x) -> p j x", j=G)
            for r in range(4):
                nc.vector.tensor_copy(
                    vv[32 * r : 32 * (r + 1), :].rearrange(
                        "p (j k) -> p j k", j=G
                    ),
                    vtp_jx[:, :, r : NC_ : 4],
                )
            vblk = sp.tile([128, G * 4 * DC], F16, tag="vblk")
            nc.gpsimd.tensor_mul(
                vblk[:].rearrange("p (j k n) -> p j k n", j=G, k=4),
                vmask_t[:].rearrange("p (j k n) -> p j k n", j=G, k=4),
                vv[:]
                .rearrange("p (j k one) -> p j k one", j=G, one=1)
                .to_broadcast([128, G, 4, DC]),
            )
            yield

    # ---- output: strip j -> row 4g+j ----
    for j in range(G):
        nc.sync.dma_start(
            out_ap[G * g + j].rearrange("(n d) -> n d", n=NC_),
            v[32 * j : 32 * j + NC_, :],
        )


def build(n_ex=E, num_devices=N_CORES):
    assert n_ex % G == 0
    nc = bacc.Bacc(
        "TRN2", target_bir_lowering=False, debug=False, num_devices=num_devices
    )
    xT_d = nc.dram_tensor("xT", [n_ex, D, S], F32R, kind="ExternalInput")
    W_d = nc.dram_tensor("W", [D, O], F32R, kind="ExternalInput")
    dmask_d = nc.dram_tensor("dmask", [128, O], F32, kind="ExternalInput")
    vmask_d = nc.dram_tensor("vmask", [128, G * 4 * DC], F16, kind="ExternalInput")
    identH_d = nc.dram_tensor("identH", [128, 128], F16, kind="ExternalInput")
    identF_d = nc.dram_tensor("identF", [128, 128], F32, kind="ExternalInput")
    ind4_d = nc.dram_tensor("ind4", [128, 4], F32, kind="ExternalInput")
    ind4T_d = nc.dram_tensor("ind4T", [4, 128], F32, kind="ExternalInput")
    out_d = nc.dram_tensor("out", [n_ex, O], F32, kind="ExternalOutput")

    with tile.TileContext(nc) as tc:
        with (
            tc.tile_pool(name="consts", bufs=1) as cp,
            tc.tile_pool(name="xp", bufs=3) as xp,
            tc.tile_pool(name="up", bufs=3) as up,
            tc.tile_pool(name="utp", bufs=3) as utp,
            tc.tile_pool(name="sp", bufs=3) as sp,
            tc.tile_pool(name="ctp", bufs=3) as ctp,
            tc.tile_pool(name="pcre", bufs=2, space=bass.MemorySpace.PSUM) as pcre,
            tc.tile_pool(name="pps", bufs=2, space=bass.MemorySpace.PSUM) as pps,
            tc.tile_pool(name="ptb", bufs=3, space=bass.MemorySpace.PSUM) as ptb,
            tc.tile_pool(name="psm", bufs=1, space=bass.MemorySpace.PSUM) as psm,
        ):
            W_t = []
            for k in range(KT_D):
                t = cp.tile([128, O], F32R, tag=f"W{k}")
                nc.sync.dma_start(t[:], W_d.ap()[128 * k : 128 * (k + 1), :])
                W_t.append(t)
            dmask_t = cp.tile([128, O], F32, tag="dmask")
            nc.sync.dma_start(dmask_t[:], dmask_d.ap())
            vmask_t = cp.tile([128, G * 4 * DC], F16, tag="vmask")
            nc.sync.dma_start(vmask_t[:], vmask_d.ap())
            identH_t = cp.tile([128, 128], F16, tag="identH")
            nc.sync.dma_start(identH_t[:], identH_d.ap())
            identF_t = cp.tile([128, 128], F32, tag="identF")
            nc.sync.dma_start(identF_t[:], identF_d.ap())
            ind4_t = cp.tile([128, 4], F32, tag="ind4")
            nc.sync.dma_start(ind4_t[:], ind4_d.ap())
            ind4T_t = cp.tile([4, 128], F32, tag="ind4T")
            nc.sync.dma_start(ind4T_t[:4, :], ind4T_d.ap())
            magic_t = cp.tile([128, NC_], U32, tag="magic")
            nc.vector.memset(magic_t[:], QMAGIC)

            pools = (xp, up, utp, sp, ctp, pcre, pps, ptb, psm)
            consts = (
                W_t, dmask_t, vmask_t, identH_t, identF_t, ind4_t, ind4T_t, magic_t
            )
            ngroups = n_ex // G

            def creation_gen(g):
                uh = [[None] * MT for _ in range(G)]
                uhT = [[None] * MT for _ in range(G)]
                box = [None]
                gen = emit_creation(nc, pools, consts, xT_d.ap(), g, uh, uhT, box)
                return gen, (uh, uhT, box)

            cgen, made = creation_gen(0)
            for _ in cgen:
                pass
            for g in range(ngroups):
                rgen = emit_routing(nc, pools, consts, out_d.ap(), g, *made)
                if g + 1 < ngroups:
                    cgen, made = creation_gen(g + 1)
                else:
                    cgen = None
                for _ in rgen:
                    if cgen is not None:
                        next(cgen, None)
                if cgen is not None:
                    for _ in cgen:
                        pass

    nc.compile()
    return nc


_cache = {}


def _get_program():
    if "nc" not in _cache:
        _cache["nc"] = build()
    return _cache["nc"]


def _run(x: np.ndarray, W: np.ndarray, **spmd_kwargs):
    x = np.asarray(x, np.float32)
    W = np.asarray(W, np.float32)
    nc = _get_program()
    xT = np.ascontiguousarray(x.transpose(0, 2, 1))  # [B, D, S]
    dmask, vmask, identH, identF, ind4, ind4T = host_consts()
    in_maps = []
    for c in range(N_CORES):
        in_maps.append(
            {
                "xT": xT[c * E : (c + 1) * E],
                "W": W,
                "dmask": dmask,
                "vmask": vmask,
                "identH": identH,
                "identF": identF,
                "ind4": ind4,
                "ind4T": ind4T,
            }
        )
    res = run_bass_kernel_spmd(
        nc, in_maps, core_ids=list(range(N_CORES)), **spmd_kwargs
    )
    out = np.concatenate([res.results[c]["out"] for c in range(N_CORES)], axis=0)
    return out, res


def kernel(x: np.ndarray, W: np.ndarray) -> np.ndarray:
    return _run(x, W)[0]
